# revision 4
# baseline (speedup 1.0000x reference)
"""AttentionResblock on 8 NeuronCores (Trainium2, Bass/Tile) — fp8 edition v2.

Sharding: query-token blocks of 512 (T_PAD=4096 = 8 x 512), two launches:
  Phase 1 (per core c): LayerNorm + Q/K/V projections (fp8 DoubleRow matmuls)
    for token rows [512c, 512c+512). Emits q8/k8 in DoubleRow-ready
    [128, 2, 512] head-pair tiles and v8 token-major, all fp8-e4m3.
    gamma/beta/bv/bc are folded into weights/residual on the host.
  Phase 2 (per core c): 16-head attention for its 512 query rows over all
    4096 keys. QK via fp8 DoubleRow (2x32 contraction). Softmax weights are
    produced uniformly via the fastexp bit trick on BOTH the Activation and
    Vector engines (kc-granular split for load balance):
      y = int8(alpha*(s+b) + SHIFT) bitcast fp8-e5m2 = 512*e^(s+b-9).
    ACT-path kcs get the bias accumulated in PSUM by an fp8 ident DR matmul
    (plane-0 diag, plane-1 zero, rhs = adjacent bias slots) and convert with
    activation(Identity, bias=SHIFT). DVE-path kcs add bias + SHIFT in the
    scalar_tensor_tensor (bias broadcast over the two heads). The scale/shift
    cancels in softmax: PV accumulates numerator and denominator (ones cols
    in the fp8 V tiles) with fp8 DoubleRow. Normalize, fp8 DR output
    projection with the f32 residual accumulated in PSUM via a bf16 ident
    matmul (diag 2^20), per-qc pipelined output DMA.
"""

import sys

sys.path.insert(0, "/opt/trn_rl_repo")

from contextlib import ExitStack  # noqa: E402

import numpy as np  # noqa: E402
import ml_dtypes  # noqa: E402

import concourse.bass as bass  # noqa: E402
import concourse.bacc as bacc  # noqa: E402
import concourse.tile as tile  # noqa: E402
from concourse import mybir  # noqa: E402
from concourse.bass_utils import run_bass_kernel_spmd  # noqa: E402
from concourse.masks import make_identity  # noqa: E402

F32 = mybir.dt.float32
BF16 = mybir.dt.bfloat16
F8E4 = mybir.dt.float8e4
F8E5 = mybir.dt.float8e5
I8 = mybir.dt.int8
AF = mybir.ActivationFunctionType
ALU = mybir.AluOpType
DR = mybir.MatmulPerfMode.DoubleRow

E4NP = ml_dtypes.float8_e4m3
E5NP = ml_dtypes.float8_e5m2
BFNP = ml_dtypes.bfloat16

N_STATE = 1024
N_HEADS = 16
D_HEAD = 64
N_CTX = 4080
T_PAD = 4096
N_CORES = 8
TOK = 512
P = 128
LN_EPS = 1e-5
NSC = 8  # state chunks of 128
NTC = 4  # token chunks per core
NKC = 32  # key chunks of 128
NKCP = 16  # key-chunk pairs of 256
NPAIR = 8  # head pairs

# fp8 scale plan
ALPHA = 4 * np.log2(np.e)  # logit scale in PSUM: psum = ALPHA*s
C_SHIFT = 9.0  # global logit shift (measured |s| max 6.5)
PMULT = 512.0  # weights premultiplier (cancels in softmax)
BETA = 96.0  # 60 + 4*log2(PMULT)
SHIFT = float(BETA - ALPHA * C_SHIFT)  # fastexp affine shift; y in [1, 86]
LAM_R = 16.0  # LN output scale
LAM_W = 512.0  # Wq/Wk/Wv scale
LAM_Q = float(np.sqrt(ALPHA / 8.0))  # q/k scales; 8*LAM_Q*LAM_K = ALPHA
LAM_V = 16.0
LAM_ATTN = 32.0
LAM_WC = 32768.0
ONES_VAL = LAM_V / LAM_ATTN  # 0.5, folded into denominator columns
GQ = LAM_Q / (LAM_R * LAM_W)
GV = LAM_V / (LAM_R * LAM_W)
G_OUT = 1.0 / (LAM_ATTN * LAM_WC)
VW = NPAIR * 130 + 32  # v8 tile width: per-pair 130 cols + tail padding

# kc -> engine split per j: 17 ACT / 15 DVE, evenly interleaved
ACT_SET = {i for i in range(NKC) if (i * 17) // NKC != ((i - 1) * 17) // NKC}
assert len(ACT_SET) == 17


def _build_phase1() -> bass.Bass:
    nc = bacc.Bacc("TRN2", target_bir_lowering=False, debug=False, num_devices=N_CORES)
    m_blk = nc.dram_tensor("m_blk", [TOK, N_STATE], BF16, kind="ExternalInput")
    Wq8 = nc.dram_tensor("Wq8", [P, 4, 2, N_STATE], F8E4, kind="ExternalInput")
    Wk8 = nc.dram_tensor("Wk8", [P, 4, 2, N_STATE], F8E4, kind="ExternalInput")
    Wv8 = nc.dram_tensor("Wv8", [P, 4, 2, N_STATE], F8E4, kind="ExternalInput")
    bqs = nc.dram_tensor("bqs", [N_STATE], F32, kind="ExternalInput")
    q8_out = nc.dram_tensor("q8_out", [4, P, 2, TOK], F8E4, kind="ExternalOutput")
    k8_out = nc.dram_tensor("k8_out", [4, P, 2, TOK], F8E4, kind="ExternalOutput")
    v8_out = nc.dram_tensor("v8_out", [TOK, N_STATE], F8E4, kind="ExternalOutput")

    with ExitStack() as ctx:
        tc = ctx.enter_context(tile.TileContext(nc))
        consts = ctx.enter_context(tc.tile_pool(name="consts", bufs=1))
        small = ctx.enter_context(tc.tile_pool(name="small", bufs=4))
        work = ctx.enter_context(tc.tile_pool(name="work", bufs=2))
        psum = ctx.enter_context(tc.tile_pool(name="psum", bufs=2, space="PSUM"))
        pst_pool = ctx.enter_context(tc.tile_pool(name="pst", bufs=2, space="PSUM"))

        identB = consts.tile([P, P], BF16)
        make_identity(nc, identB)
        eps_sb = consts.tile([P, 1], F32)
        nc.vector.memset(eps_sb, LN_EPS / (LAM_R * LAM_R))
        bqs_sb = consts.tile([P, NSC], F32)

        m_sb = consts.tile([P, NTC, N_STATE], BF16)
        w_sb = {}
        for name, w in (("Wq8", Wq8), ("Wk8", Wk8), ("Wv8", Wv8)):
            w_sb[name] = consts.tile([P, 4, 2, N_STATE], F8E4, name=f"{name}_sb")

        def ld_m(tcn):
            nc.sync.dma_start(
                out=m_sb[:, tcn, :],
                in_=m_blk.rearrange("(c p) s -> p c s", p=P)[:, tcn, :],
            )

        ld_m(0)
        ld_m(1)
        ld_m(2)
        ld_m(3)
        nc.sync.dma_start(out=bqs_sb, in_=bqs.rearrange("(j p) -> p j", p=P))
        nc.sync.dma_start(out=w_sb["Wq8"], in_=Wq8[:, :, :, :])
        nc.sync.dma_start(out=w_sb["Wk8"], in_=Wk8[:, :, :, :])
        nc.sync.dma_start(out=w_sb["Wv8"], in_=Wv8[:, :, :, :])

        # LayerNorm -> xcB = (m - mu) * rstd * LAM_R in bf16
        # stats via bn_stats/bn_aggr (single DVE pass), rstd via ACT sqrt
        xcB = consts.tile([P, NTC, N_STATE], BF16)
        for tcn in range(NTC):
            bns = small.tile([P, 2, 6], F32, tag="bns")
            nc.vector.bn_stats(bns[:, 0, :], m_sb[:, tcn, 0 : N_STATE // 2])
            nc.vector.bn_stats(bns[:, 1, :], m_sb[:, tcn, N_STATE // 2 :])
            agg = small.tile([P, 2], F32, tag="agg")
            nc.vector.bn_aggr(agg, bns)
            std = small.tile([P, 1], F32, tag="std")
            nc.scalar.activation(
                out=std, in_=agg[:, 1:2], func=AF.Sqrt, bias=eps_sb,
                scale=1.0 / (LAM_R * LAM_R),
            )
            rstdl = small.tile([P, 1], F32, tag="rstdl")
            nc.vector.reciprocal(rstdl, std)
            nc.vector.tensor_scalar(
                out=xcB[:, tcn, :],
                in0=m_sb[:, tcn, :],
                scalar1=agg[:, 0:1],
                scalar2=rstdl,
                op0=ALU.subtract,
                op1=ALU.mult,
            )

        # transpose to state-major and quantize: rT8 [128, sc, 512] e4m3
        rT8 = consts.tile([P, NSC, TOK], F8E4)
        for scp in range(NSC // 2):
            pst = pst_pool.tile([P, 2, TOK], BF16, tag="pst")
            for half in range(2):
                sc = 2 * scp + half
                for tcn in range(NTC):
                    nc.tensor.transpose(
                        pst[:, half, tcn * P : (tcn + 1) * P],
                        xcB[:, tcn, sc * P : (sc + 1) * P],
                        identB,
                    )
            if scp % 2 == 0:
                nc.vector.tensor_copy(rT8[:, 2 * scp : 2 * scp + 2, :], pst)
            else:
                nc.scalar.copy(rT8[:, 2 * scp : 2 * scp + 2, :], pst)

        # q/k: DoubleRow fp8 matmuls, evacuate into [128, 2, 512] pair tiles
        q8g = [consts.tile([P, 2, TOK], F8E4, name=f"q8g{g}") for g in range(4)]
        k8g = [consts.tile([P, 2, TOK], F8E4, name=f"k8g{g}") for g in range(4)]
        v8sb = consts.tile([P, NTC, N_STATE], F8E4)

        def emit_v(tcn):
            psv = psum.tile([P, N_STATE], F32, tag="psv", bufs=1)
            for pc in range(2):
                for s in range(4):
                    nc.tensor.matmul(
                        psv[:, pc * TOK : (pc + 1) * TOK],
                        lhsT=rT8[:, 2 * s : 2 * s + 2, tcn * P : (tcn + 1) * P],
                        rhs=w_sb["Wv8"][:, s, :, pc * TOK : (pc + 1) * TOK],
                        start=(s == 0),
                        stop=(s == 3),
                        perf_mode=DR,
                    )
            if tcn % 2 == 0:
                nc.scalar.mul(v8sb[:, tcn, :], psv, GV)
            else:
                nc.vector.tensor_scalar(
                    out=v8sb[:, tcn, :], in0=psv, scalar1=GV, scalar2=None,
                    op0=ALU.mult,
                )
            nc.sync.dma_start(
                out=v8_out.rearrange("(c p) s -> p c s", p=P)[:, tcn, :],
                in_=v8sb[:, tcn, :],
            )

        for j in range(NSC):
            g, half = j // 2, j % 2
            psq = psum.tile([P, TOK], F32, tag="psq")
            psk = psum.tile([P, TOK], F32, tag="psk")
            for s in range(4):
                nc.tensor.matmul(
                    psq,
                    lhsT=w_sb["Wq8"][:, s, :, j * P : (j + 1) * P],
                    rhs=rT8[:, 2 * s : 2 * s + 2, :],
                    start=(s == 0),
                    stop=(s == 3),
                    perf_mode=DR,
                )
            for s in range(4):
                nc.tensor.matmul(
                    psk,
                    lhsT=w_sb["Wk8"][:, s, :, j * P : (j + 1) * P],
                    rhs=rT8[:, 2 * s : 2 * s + 2, :],
                    start=(s == 0),
                    stop=(s == 3),
                    perf_mode=DR,
                )
            for t in range(2):
                nc.scalar.activation(
                    out=q8g[g][64 * half : 64 * half + 64, t, :],
                    in_=psq[64 * t : 64 * t + 64, :],
                    func=AF.Identity,
                    bias=bqs_sb[64 * t : 64 * t + 64, j : j + 1],
                    scale=GQ,
                )
            # split k evac across both engines for balance
            nc.vector.tensor_scalar(
                out=k8g[g][64 * half : 64 * half + 64, 0, :],
                in0=psk[0:64, :],
                scalar1=GQ,
                scalar2=None,
                op0=ALU.mult,
            )
            nc.scalar.mul(
                k8g[g][64 * half : 64 * half + 64, 1, :], psk[64:128, :], GQ
            )
            if j % 2 == 1:
                emit_v(j // 2)
                nc.sync.dma_start(out=q8_out[g, :, :, :], in_=q8g[g])
                nc.sync.dma_start(out=k8_out[g, :, :, :], in_=k8g[g])
    nc.compile()
    return nc


def _build_phase2() -> bass.Bass:
    nc = bacc.Bacc("TRN2", target_bir_lowering=False, debug=False, num_devices=N_CORES)
    q8_in = nc.dram_tensor("q8_in", [4, P, 2, TOK], F8E4, kind="ExternalInput")
    k8_in = nc.dram_tensor("k8_in", [4, P, 2, T_PAD], F8E4, kind="ExternalInput")
    v8_in = nc.dram_tensor("v8_in", [4, P, NKC // 4, VW], F8E4, kind="ExternalInput")
    b8_in = nc.dram_tensor("b8_in", [NKC + 1, P, TOK], F8E4, kind="ExternalInput")
    mres = nc.dram_tensor("mres", [TOK, N_STATE], BF16, kind="ExternalInput")
    Wc8 = nc.dram_tensor("Wc8", [P, 4, 2, N_STATE], F8E4, kind="ExternalInput")
    o_out = nc.dram_tensor("o_out", [TOK, N_STATE], BF16, kind="ExternalOutput")

    with ExitStack() as ctx:
        tc = ctx.enter_context(tile.TileContext(nc))
        consts = ctx.enter_context(tc.tile_pool(name="consts", bufs=1))
        small = ctx.enter_context(tc.tile_pool(name="small", bufs=2))
        ppool = ctx.enter_context(tc.tile_pool(name="ppool", bufs=4))
        psqk = ctx.enter_context(tc.tile_pool(name="psqk", bufs=3, space="PSUM"))
        pspv = ctx.enter_context(tc.tile_pool(name="pspv", bufs=1, space="PSUM"))

        # ident for bias injection: plane 0 = diag(1), plane 1 = 0
        identg = consts.tile([P, 2, P], F8E4)
        nc.vector.memset(identg, 0.0)
        make_identity(nc, identg[:, 0, :])
        shift_ap = consts.tile([P, 1], F32)
        nc.vector.memset(shift_ap, SHIFT)

        q8sb = consts.tile([P, 4, 2, TOK], F8E4)
        k8sb = consts.tile([P, 4, 2, T_PAD], F8E4)
        v8sb = consts.tile([P, NKC, VW], F8E4)
        b8sb = consts.tile([P, NKC + 1, TOK], F8E4)
        m_sb = consts.tile([P, NTC, N_STATE], BF16)
        wc_sb = consts.tile([P, 4, 2, N_STATE], F8E4)

        def ld_q8(g):
            nc.sync.dma_start(out=q8sb[:, g, :, :], in_=q8_in[g, :, :, :])

        def ld_k8(g, half=None):
            if half is None:
                nc.sync.dma_start(out=k8sb[:, g, :, :], in_=k8_in[g, :, :, :])
            else:
                sl = slice(half * (T_PAD // 2), (half + 1) * (T_PAD // 2))
                nc.sync.dma_start(out=k8sb[:, g, :, sl], in_=k8_in[g, :, :, sl])

        def ld_b8(i0, i1):
            nc.sync.dma_start(
                out=b8sb[:, i0:i1, :],
                in_=b8_in[i0:i1, :, :].rearrange("k p t -> p k t"),
            )

        def ld_v8(q):
            nc.sync.dma_start(
                out=v8sb[:, q * 8 : (q + 1) * 8, :], in_=v8_in[q, :, :, :]
            )

        ld_q8(0)
        ld_k8(0, 0)
        ld_b8(0, 9)
        ld_k8(0, 1)
        ld_b8(9, 17)
        ld_v8(0)
        ld_b8(17, 25)
        ld_v8(1)
        ld_b8(25, 33)
        ld_v8(2)
        ld_k8(1)
        ld_v8(3)
        ld_q8(1)
        ld_k8(2)
        ld_q8(2)
        ld_k8(3)
        ld_q8(3)
        nc.sync.dma_start(out=m_sb, in_=mres.rearrange("(c p) s -> p c s", p=P))
        nc.sync.dma_start(out=wc_sb, in_=Wc8[:, :, :, :])

        # residual ident: diag = 1/G_OUT = 2^20 (exact in bf16)
        identB = consts.tile([P, P], BF16)
        make_identity(nc, identB)
        identR = consts.tile([P, P], BF16)
        nc.scalar.mul(identR, identB, 1.0 / G_OUT)

        attnT8 = consts.tile([P, NPAIR, TOK], F8E4)

        for j in range(NPAIR):
            g, half = j // 2, j % 2
            pv = pspv.tile([96, 2, TOK], F32, tag="pv")
            for kcp in range(NKCP):
                ptile = ppool.tile([P, 2, 2, TOK], F8E5, tag="pt")
                for sub in range(2):
                    kc = 2 * kcp + sub
                    is_act = kc in ACT_SET
                    ps = psqk.tile([P, 2, TOK], F32, tag="s")
                    for h in range(2):
                        base = 64 * half + 32 * h
                        nc.tensor.matmul(
                            ps[:, h, :],
                            lhsT=k8sb[base : base + 32, g, :, kc * P : (kc + 1) * P],
                            rhs=q8sb[base : base + 32, g, :, :],
                            start=True,
                            stop=not is_act,
                            perf_mode=DR,
                            tile_position=(base, 0),
                        )
                        if is_act:
                            nc.tensor.matmul(
                                ps[:, h, :],
                                lhsT=identg,
                                rhs=b8sb[:, kc : kc + 2, :],
                                start=False,
                                stop=True,
                                perf_mode=DR,
                                skip_group_check=True,
                            )
                    if is_act:
                        nc.scalar.activation(
                            out=ptile[:, :, sub, :].bitcast(I8),
                            in_=ps,
                            func=AF.Identity,
                            bias=shift_ap,
                            scale=1.0,
                        )
                    else:
                        nc.vector.scalar_tensor_tensor(
                            out=ptile[:, :, sub, :].bitcast(I8),
                            in0=ps,
                            scalar=SHIFT,
                            in1=b8sb[:, kc : kc + 1, :].rearrange(
                                "p (o k) n -> p o k n", o=1
                            )[:, 0, :, :].broadcast_to([P, 2, TOK]),
                            op0=ALU.add,
                            op1=ALU.add,
                        )
                nc.tensor.matmul(
                    pv[:, 0, :],
                    lhsT=v8sb[:, 2 * kcp : 2 * kcp + 2, 130 * j : 130 * j + 96],
                    rhs=ptile[:, 0, :, :],
                    start=(kcp == 0),
                    stop=(kcp == NKCP - 1),
                    perf_mode=DR,
                )
                nc.tensor.matmul(
                    pv[:, 1, :],
                    lhsT=v8sb[:, 2 * kcp : 2 * kcp + 2, 130 * j + 65 : 130 * j + 161],
                    rhs=ptile[:, 1, :, :],
                    start=(kcp == 0),
                    stop=(kcp == NKCP - 1),
                    perf_mode=DR,
                )

            # evac pv (both halves in one op), reciprocal of denominators,
            # broadcast + normalize off the critical path on Pool (DVE for j=7)
            st = small.tile([65, 2, TOK], BF16, tag="st")
            nc.scalar.copy(st, pv[0:65, :, :])
            rec = small.tile([1, 2, TOK], BF16, tag="rec")
            with nc.allow_low_precision("bf16 softmax denominators, ~0.4% scale"):
                nc.vector.reciprocal(rec, st[64:65, :, :])
            bcastA = small.tile([64, TOK], BF16, tag="bcastA")
            bcastB = small.tile([64, TOK], BF16, tag="bcastB")
            nc.gpsimd.partition_broadcast(bcastA, rec[:, 0, :], channels=64)
            nc.gpsimd.partition_broadcast(bcastB, rec[:, 1, :], channels=64)
            eng = nc.vector if j == NPAIR - 1 else nc.gpsimd
            eng.tensor_tensor(
                out=attnT8[0:64, j, :], in0=st[0:64, 0, :], in1=bcastA,
                op=ALU.mult,
            )
            eng.tensor_tensor(
                out=attnT8[64:128, j, :], in0=st[0:64, 1, :], in1=bcastB,
                op=ALU.mult,
            )

        # output projection (fp8 DR) with residual accumulated via bf16 ident
        o_sb = consts.tile([P, NTC, N_STATE], BF16)
        for qc in range(NTC):
            ps_o = psqk.tile([P, 2, TOK], F32, tag="s")
            po = ps_o.rearrange("p a b -> p (a b)")
            for pc in range(2):
                for u in range(4):
                    nc.tensor.matmul(
                        ps_o[:, pc, :],
                        lhsT=attnT8[:, 2 * u : 2 * u + 2, qc * P : (qc + 1) * P],
                        rhs=wc_sb[:, u, :, pc * TOK : (pc + 1) * TOK],
                        start=(u == 0),
                        stop=False,
                        perf_mode=DR,
                    )
                nc.tensor.matmul(
                    ps_o[:, pc, :],
                    lhsT=identR,
                    rhs=m_sb[:, qc, pc * TOK : (pc + 1) * TOK],
                    start=False,
                    stop=True,
                    skip_group_check=True,
                )
            if qc % 2 == 0:
                nc.scalar.mul(o_sb[:, qc, :], po, G_OUT)
            else:
                nc.vector.tensor_scalar(
                    out=o_sb[:, qc, :], in0=po, scalar1=G_OUT, scalar2=None,
                    op0=ALU.mult,
                )
            nc.sync.dma_start(
                out=o_out.rearrange("(c p) s -> p c s", p=P)[:, qc, :],
                in_=o_sb[:, qc, :],
            )
    nc.compile()
    return nc


_NC_CACHE = {}


def _get_nc(which):
    if which not in _NC_CACHE:
        _NC_CACHE[which] = _build_phase1() if which == 1 else _build_phase2()
    return _NC_CACHE[which]


def _perm_cols():
    """Column permutation for q/k weights: per pair j, [hA d0:32 | hB d0:32 |
    hA d32:64 | hB d32:64]."""
    order = []
    for j in range(NSC):
        hA, hB = 2 * j, 2 * j + 1
        order.extend(range(hA * 64, hA * 64 + 32))
        order.extend(range(hB * 64, hB * 64 + 32))
        order.extend(range(hA * 64 + 32, hA * 64 + 64))
        order.extend(range(hB * 64 + 32, hB * 64 + 64))
    return np.array(order)


def _w_dr_layout(w8):
    """[1024, C] -> [128, 4, 2, C] DoubleRow lhsT layout."""
    return np.ascontiguousarray(
        w8.reshape(4, 2, P, -1).transpose(2, 0, 1, 3)
    )


def kernel(m, bias, gamma, beta, Wq, bq, Wk, Wv, bv, Wc, bc, _want_timing=None):
    m = np.asarray(m, dtype=np.float32).reshape(N_CTX, N_STATE)
    bias = np.asarray(bias, np.float32)
    gamma = np.asarray(gamma, np.float32)
    beta = np.asarray(beta, np.float32)
    Wq = np.asarray(Wq, np.float32)
    Wk = np.asarray(Wk, np.float32)
    Wv = np.asarray(Wv, np.float32)
    Wc = np.asarray(Wc, np.float32)
    bq = np.asarray(bq, np.float32)
    bv = np.asarray(bv, np.float32)
    bc = np.asarray(bc, np.float32)

    m_pad = np.zeros((T_PAD, N_STATE), np.float32)
    m_pad[:N_CTX] = m

    # fold gamma into weights, beta into biases; bv and bc fold into residual
    Wqf = gamma[:, None] * Wq
    Wkf = gamma[:, None] * Wk
    Wvf = gamma[:, None] * Wv
    bqf = bq + beta @ Wq
    # beta@Wk shifts all logits of a query equally -> softmax invariant; drop.
    # beta@Wv + bv shift attention output -> fold into residual with bc.
    perm = _perm_cols()
    Wq8 = _w_dr_layout((LAM_W * Wqf[:, perm]).astype(E4NP))
    Wk8 = _w_dr_layout((LAM_W * Wkf[:, perm]).astype(E4NP))
    Wv8 = _w_dr_layout((LAM_W * Wvf).astype(E4NP))
    bqs = (LAM_Q * bqf[perm]).astype(np.float32)
    Wc8 = _w_dr_layout((LAM_WC * Wc).astype(E4NP))
    mres_full = (m_pad + (bc + (bv + beta @ Wv) @ Wc)[None, :]).astype(BFNP)

    import sys as _sys

    def _log(*a):
        print("[kernel]", *a, file=_sys.stderr, flush=True)

    nc1 = _get_nc(1)
    _log("phase1 built")
    in_maps1 = []
    for c in range(N_CORES):
        in_maps1.append(
            {
                "m_blk": np.ascontiguousarray(
                    m_pad[c * TOK : (c + 1) * TOK].astype(BFNP)
                ),
                "Wq8": Wq8,
                "Wk8": Wk8,
                "Wv8": Wv8,
                "bqs": bqs,
            }
        )
    res1 = run_bass_kernel_spmd(nc1, in_maps1, core_ids=list(range(N_CORES)))
    _log("phase1 done")

    q8_blks = [r["q8_out"] for r in res1.results]
    k8_full = np.concatenate([r["k8_out"] for r in res1.results], axis=3)
    v8_full = np.concatenate([r["v8_out"] for r in res1.results], axis=0)
    v8_full[N_CTX:] = 0  # pad tokens carry no value

    # v8 pair-tile layout [128, 32, VW] with denominator columns
    v8f = v8_full.astype(np.float32).reshape(NKC, P, N_HEADS, D_HEAD)
    v8h = np.zeros((P, NKC, VW), np.float32)
    for j in range(NPAIR):
        v8h[:, :, 130 * j : 130 * j + 64] = v8f[:, :, 2 * j].transpose(1, 0, 2)
        v8h[:, :, 130 * j + 65 : 130 * j + 129] = v8f[:, :, 2 * j + 1].transpose(1, 0, 2)
        v8h[:, :, 130 * j + 64] = ONES_VAL
        v8h[:, :, 130 * j + 129] = ONES_VAL
    # zero the denominator contribution of padded keys
    keyidx = (np.arange(NKC)[None, :] * P + np.arange(P)[:, None])  # [p, kc]
    padmask = keyidx >= N_CTX
    for j in range(NPAIR):
        v8h[:, :, 130 * j + 64][padmask] = 0.0
        v8h[:, :, 130 * j + 129][padmask] = 0.0
    v8h8 = v8h.astype(E4NP)
    v8_dr = np.ascontiguousarray(
        v8h8.reshape(P, 4, NKC // 4, VW).transpose(1, 0, 2, 3)
    )

    biasT = np.ascontiguousarray(bias.T)  # [k, q]

    nc2 = _get_nc(2)
    _log("phase2 built")
    in_maps2 = []
    for c in range(N_CORES):
        qs = slice(c * TOK, (c + 1) * TOK)
        b8 = np.zeros((NKC + 1, P, TOK), E4NP)
        b8[:NKC] = (ALPHA * biasT[:, qs]).reshape(NKC, P, TOK).astype(E4NP)
        in_maps2.append(
            {
                "q8_in": np.ascontiguousarray(q8_blks[c]),
                "k8_in": k8_full,
                "v8_in": v8_dr,
                "b8_in": b8,
                "mres": np.ascontiguousarray(mres_full[qs]),
                "Wc8": Wc8,
            }
        )
    res2 = run_bass_kernel_spmd(nc2, in_maps2, core_ids=list(range(N_CORES)))
    _log("phase2 done")
    o = np.concatenate([r["o_out"] for r in res2.results], axis=0)[:N_CTX]
    if _want_timing is not None:
        _want_timing["res1"] = res1
        _want_timing["res2"] = res2
    return o.reshape(1, N_CTX, N_STATE).astype(np.float32)


# revision 6
# speedup vs baseline: 1.0341x; 1.0341x over previous
"""AttentionResblock on 8 NeuronCores (Trainium2, Bass/Tile) — fp8 edition v2.

Sharding: query-token blocks of 512 (T_PAD=4096 = 8 x 512), two launches:
  Phase 1 (per core c): LayerNorm + Q/K/V projections (fp8 DoubleRow matmuls)
    for token rows [512c, 512c+512). Emits q8/k8 in DoubleRow-ready
    [128, 2, 512] head-pair tiles and v8 token-major, all fp8-e4m3.
    gamma/beta/bv/bc are folded into weights/residual on the host.
  Phase 2 (per core c): 16-head attention for its 512 query rows over all
    4096 keys. QK via fp8 DoubleRow (2x32 contraction). Softmax weights are
    produced uniformly via the fastexp bit trick on BOTH the Activation and
    Vector engines (kc-granular split for load balance):
      y = int8(alpha*(s+b) + SHIFT) bitcast fp8-e5m2 = 512*e^(s+b-9).
    ACT-path kcs get the bias accumulated in PSUM by an fp8 ident DR matmul
    (plane-0 diag, plane-1 zero, rhs = adjacent bias slots) and convert with
    activation(Identity, bias=SHIFT). DVE-path kcs add bias + SHIFT in the
    scalar_tensor_tensor (bias broadcast over the two heads). The scale/shift
    cancels in softmax: PV accumulates numerator and denominator (ones cols
    in the fp8 V tiles) with fp8 DoubleRow. Normalize, fp8 DR output
    projection with the f32 residual accumulated in PSUM via a bf16 ident
    matmul (diag 2^20), per-qc pipelined output DMA.
"""

import sys

sys.path.insert(0, "/opt/trn_rl_repo")

from contextlib import ExitStack  # noqa: E402

import numpy as np  # noqa: E402
import ml_dtypes  # noqa: E402

import concourse.bass as bass  # noqa: E402
import concourse.bacc as bacc  # noqa: E402
import concourse.tile as tile  # noqa: E402
from concourse import mybir  # noqa: E402
from concourse.bass_utils import run_bass_kernel_spmd  # noqa: E402
from concourse.masks import make_identity  # noqa: E402

F32 = mybir.dt.float32
BF16 = mybir.dt.bfloat16
F8E4 = mybir.dt.float8e4
F8E5 = mybir.dt.float8e5
I8 = mybir.dt.int8
AF = mybir.ActivationFunctionType
ALU = mybir.AluOpType
DR = mybir.MatmulPerfMode.DoubleRow

E4NP = ml_dtypes.float8_e4m3
E5NP = ml_dtypes.float8_e5m2
BFNP = ml_dtypes.bfloat16

N_STATE = 1024
N_HEADS = 16
D_HEAD = 64
N_CTX = 4080
T_PAD = 4096
N_CORES = 8
TOK = 512
P = 128
LN_EPS = 1e-5
NSC = 8  # state chunks of 128
NTC = 4  # token chunks per core
NKC = 32  # key chunks of 128
NKCP = 16  # key-chunk pairs of 256
NPAIR = 8  # head pairs

# fp8 scale plan
ALPHA = 4 * np.log2(np.e)  # logit scale in PSUM: psum = ALPHA*s
C_SHIFT = 9.0  # global logit shift (measured |s| max 6.5)
PMULT = 512.0  # weights premultiplier (cancels in softmax)
BETA = 96.0  # 60 + 4*log2(PMULT)
SHIFT = float(BETA - ALPHA * C_SHIFT)  # fastexp affine shift; y in [1, 86]
LAM_R = 16.0  # LN output scale
LAM_W = 512.0  # Wq/Wk/Wv scale
LAM_Q = float(np.sqrt(ALPHA / 8.0))  # q/k scales; 8*LAM_Q*LAM_K = ALPHA
LAM_V = 16.0
LAM_ATTN = 32.0
LAM_WC = 32768.0
ONES_VAL = LAM_V / LAM_ATTN  # 0.5, folded into denominator columns
GQ = LAM_Q / (LAM_R * LAM_W)
GV = LAM_V / (LAM_R * LAM_W)
G_OUT = 1.0 / (LAM_ATTN * LAM_WC)
VW = NPAIR * 130 + 32  # v8 tile width: per-pair 130 cols + tail padding

# kc -> engine split per j: 17 ACT / 15 DVE, evenly interleaved
ACT_SET = {i for i in range(NKC) if (i * 17) // NKC != ((i - 1) * 17) // NKC}
assert len(ACT_SET) == 17


def _build_phase1() -> bass.Bass:
    nc = bacc.Bacc("TRN2", target_bir_lowering=False, debug=False, num_devices=N_CORES)
    m_blk = nc.dram_tensor("m_blk", [TOK, N_STATE], BF16, kind="ExternalInput")
    Wq8 = nc.dram_tensor("Wq8", [P, 4, 2, N_STATE], F8E4, kind="ExternalInput")
    Wk8 = nc.dram_tensor("Wk8", [P, 4, 2, N_STATE], F8E4, kind="ExternalInput")
    Wv8 = nc.dram_tensor("Wv8", [P, 4, 2, N_STATE], F8E4, kind="ExternalInput")
    bqs = nc.dram_tensor("bqs", [N_STATE], F32, kind="ExternalInput")
    q8_out = nc.dram_tensor("q8_out", [4, P, 2, TOK], F8E4, kind="ExternalOutput")
    k8_out = nc.dram_tensor("k8_out", [4, P, 2, TOK], F8E4, kind="ExternalOutput")
    v8_out = nc.dram_tensor("v8_out", [TOK, N_STATE], F8E4, kind="ExternalOutput")

    with ExitStack() as ctx:
        tc = ctx.enter_context(tile.TileContext(nc))
        consts = ctx.enter_context(tc.tile_pool(name="consts", bufs=1))
        small = ctx.enter_context(tc.tile_pool(name="small", bufs=4))
        work = ctx.enter_context(tc.tile_pool(name="work", bufs=2))
        psum = ctx.enter_context(tc.tile_pool(name="psum", bufs=2, space="PSUM"))
        pst_pool = ctx.enter_context(tc.tile_pool(name="pst", bufs=2, space="PSUM"))

        identB = consts.tile([P, P], BF16)
        make_identity(nc, identB)
        eps_sb = consts.tile([P, 1], F32)
        nc.vector.memset(eps_sb, LN_EPS / (LAM_R * LAM_R))
        bqs_sb = consts.tile([P, NSC], F32)

        m_sb = consts.tile([P, NTC, N_STATE], BF16)
        w_sb = {}
        for name, w in (("Wq8", Wq8), ("Wk8", Wk8), ("Wv8", Wv8)):
            w_sb[name] = consts.tile([P, 4, 2, N_STATE], F8E4, name=f"{name}_sb")

        def ld_m(tcn):
            nc.sync.dma_start(
                out=m_sb[:, tcn, :],
                in_=m_blk.rearrange("(c p) s -> p c s", p=P)[:, tcn, :],
            )

        ld_m(0)
        ld_m(1)
        ld_m(2)
        ld_m(3)
        nc.sync.dma_start(out=bqs_sb, in_=bqs.rearrange("(j p) -> p j", p=P))
        nc.sync.dma_start(out=w_sb["Wq8"], in_=Wq8[:, :, :, :])
        nc.sync.dma_start(out=w_sb["Wk8"], in_=Wk8[:, :, :, :])
        nc.sync.dma_start(out=w_sb["Wv8"], in_=Wv8[:, :, :, :])

        # LayerNorm -> xcB = (m - mu) * rstd * LAM_R in bf16
        # stats via bn_stats/bn_aggr (single DVE pass), rstd via ACT sqrt
        xcB = consts.tile([P, NTC, N_STATE], BF16)
        for tcn in range(NTC):
            bns = small.tile([P, 2, 6], F32, tag="bns")
            nc.vector.bn_stats(bns[:, 0, :], m_sb[:, tcn, 0 : N_STATE // 2])
            nc.vector.bn_stats(bns[:, 1, :], m_sb[:, tcn, N_STATE // 2 :])
            agg = small.tile([P, 2], F32, tag="agg")
            nc.vector.bn_aggr(agg, bns)
            std = small.tile([P, 1], F32, tag="std")
            nc.scalar.activation(
                out=std, in_=agg[:, 1:2], func=AF.Sqrt, bias=eps_sb,
                scale=1.0 / (LAM_R * LAM_R),
            )
            rstdl = small.tile([P, 1], F32, tag="rstdl")
            nc.vector.reciprocal(rstdl, std)
            nc.vector.tensor_scalar(
                out=xcB[:, tcn, :],
                in0=m_sb[:, tcn, :],
                scalar1=agg[:, 0:1],
                scalar2=rstdl,
                op0=ALU.subtract,
                op1=ALU.mult,
            )

        # transpose to state-major and quantize: rT8 [128, sc, 512] e4m3
        rT8 = consts.tile([P, NSC, TOK], F8E4)
        for scp in range(NSC // 2):
            pst = pst_pool.tile([P, 2, TOK], BF16, tag="pst")
            for half in range(2):
                sc = 2 * scp + half
                for tcn in range(NTC):
                    nc.tensor.transpose(
                        pst[:, half, tcn * P : (tcn + 1) * P],
                        xcB[:, tcn, sc * P : (sc + 1) * P],
                        identB,
                    )
            if scp % 2 == 0:
                nc.vector.tensor_copy(rT8[:, 2 * scp : 2 * scp + 2, :], pst)
            else:
                nc.scalar.copy(rT8[:, 2 * scp : 2 * scp + 2, :], pst)

        # q/k: DoubleRow fp8 matmuls, evacuate into [128, 2, 512] pair tiles
        q8g = [consts.tile([P, 2, TOK], F8E4, name=f"q8g{g}") for g in range(4)]
        k8g = [consts.tile([P, 2, TOK], F8E4, name=f"k8g{g}") for g in range(4)]
        v8sb = consts.tile([P, NTC, N_STATE], F8E4)

        def emit_v(tcn):
            psv = psum.tile([P, N_STATE], F32, tag="psv", bufs=1)
            for pc in range(2):
                for s in range(4):
                    nc.tensor.matmul(
                        psv[:, pc * TOK : (pc + 1) * TOK],
                        lhsT=rT8[:, 2 * s : 2 * s + 2, tcn * P : (tcn + 1) * P],
                        rhs=w_sb["Wv8"][:, s, :, pc * TOK : (pc + 1) * TOK],
                        start=(s == 0),
                        stop=(s == 3),
                        perf_mode=DR,
                    )
            if tcn % 2 == 0:
                nc.scalar.mul(v8sb[:, tcn, :], psv, GV)
            else:
                nc.vector.tensor_scalar(
                    out=v8sb[:, tcn, :], in0=psv, scalar1=GV, scalar2=None,
                    op0=ALU.mult,
                )
            nc.sync.dma_start(
                out=v8_out.rearrange("(c p) s -> p c s", p=P)[:, tcn, :],
                in_=v8sb[:, tcn, :],
            )

        for j in range(NSC):
            g, half = j // 2, j % 2
            psq = psum.tile([P, TOK], F32, tag="psq")
            psk = psum.tile([P, TOK], F32, tag="psk")
            for s in range(4):
                nc.tensor.matmul(
                    psq,
                    lhsT=w_sb["Wq8"][:, s, :, j * P : (j + 1) * P],
                    rhs=rT8[:, 2 * s : 2 * s + 2, :],
                    start=(s == 0),
                    stop=(s == 3),
                    perf_mode=DR,
                )
            for s in range(4):
                nc.tensor.matmul(
                    psk,
                    lhsT=w_sb["Wk8"][:, s, :, j * P : (j + 1) * P],
                    rhs=rT8[:, 2 * s : 2 * s + 2, :],
                    start=(s == 0),
                    stop=(s == 3),
                    perf_mode=DR,
                )
            for t in range(2):
                nc.scalar.activation(
                    out=q8g[g][64 * half : 64 * half + 64, t, :],
                    in_=psq[64 * t : 64 * t + 64, :],
                    func=AF.Identity,
                    bias=bqs_sb[64 * t : 64 * t + 64, j : j + 1],
                    scale=GQ,
                )
            # split k evac across both engines for balance
            nc.vector.tensor_scalar(
                out=k8g[g][64 * half : 64 * half + 64, 0, :],
                in0=psk[0:64, :],
                scalar1=GQ,
                scalar2=None,
                op0=ALU.mult,
            )
            nc.scalar.mul(
                k8g[g][64 * half : 64 * half + 64, 1, :], psk[64:128, :], GQ
            )
            if j % 2 == 1:
                emit_v(j // 2)
                nc.sync.dma_start(out=q8_out[g, :, :, :], in_=q8g[g])
                nc.sync.dma_start(out=k8_out[g, :, :, :], in_=k8g[g])
    nc.compile()
    return nc


def _build_phase2() -> bass.Bass:
    nc = bacc.Bacc("TRN2", target_bir_lowering=False, debug=False, num_devices=N_CORES)
    q8_in = nc.dram_tensor("q8_in", [4, P, 2, TOK], F8E4, kind="ExternalInput")
    k8_in = nc.dram_tensor("k8_in", [4, P, 2, T_PAD], F8E4, kind="ExternalInput")
    v8_in = nc.dram_tensor("v8_in", [4, P, NKC // 4, VW], F8E4, kind="ExternalInput")
    b8_in = nc.dram_tensor("b8_in", [NKC + 1, P, TOK], F8E4, kind="ExternalInput")
    mres = nc.dram_tensor("mres", [TOK, N_STATE], BF16, kind="ExternalInput")
    Wc8 = nc.dram_tensor("Wc8", [P, 4, 2, N_STATE], F8E4, kind="ExternalInput")
    o_out = nc.dram_tensor("o_out", [TOK, N_STATE], BF16, kind="ExternalOutput")

    with ExitStack() as ctx:
        tc = ctx.enter_context(tile.TileContext(nc))
        consts = ctx.enter_context(tc.tile_pool(name="consts", bufs=1))
        small = ctx.enter_context(tc.tile_pool(name="small", bufs=2))
        ppool = ctx.enter_context(tc.tile_pool(name="ppool", bufs=4))
        psqk = ctx.enter_context(tc.tile_pool(name="psqk", bufs=3, space="PSUM"))
        pspv = ctx.enter_context(tc.tile_pool(name="pspv", bufs=1, space="PSUM"))

        # ident for bias injection: plane 0 = diag(1), plane 1 = 0
        identg = consts.tile([P, 2, P], F8E4)
        nc.vector.memset(identg, 0.0)
        make_identity(nc, identg[:, 0, :])
        shift_ap = consts.tile([P, 1], F32)
        nc.vector.memset(shift_ap, SHIFT)

        q8sb = consts.tile([P, 4, 2, TOK], F8E4)
        k8sb = consts.tile([P, 4, 2, T_PAD], F8E4)
        v8sb = consts.tile([P, NKC, VW], F8E4)
        b8sb = consts.tile([P, NKC + 1, TOK], F8E4)
        m_sb = consts.tile([P, NTC, N_STATE], BF16)
        wc_sb = consts.tile([P, 4, 2, N_STATE], F8E4)

        def ld_q8(g):
            nc.sync.dma_start(out=q8sb[:, g, :, :], in_=q8_in[g, :, :, :])

        def ld_k8(g, half=None):
            if half is None:
                nc.sync.dma_start(out=k8sb[:, g, :, :], in_=k8_in[g, :, :, :])
            else:
                sl = slice(half * (T_PAD // 2), (half + 1) * (T_PAD // 2))
                nc.sync.dma_start(out=k8sb[:, g, :, sl], in_=k8_in[g, :, :, sl])

        def ld_b8(i0, i1):
            nc.sync.dma_start(
                out=b8sb[:, i0:i1, :],
                in_=b8_in[i0:i1, :, :].rearrange("k p t -> p k t"),
            )

        def ld_v8(q):
            nc.sync.dma_start(
                out=v8sb[:, q * 8 : (q + 1) * 8, :], in_=v8_in[q, :, :, :]
            )

        ld_q8(0)
        ld_k8(0, 0)
        ld_b8(0, 9)
        ld_k8(0, 1)
        ld_b8(9, 17)
        ld_v8(0)
        ld_b8(17, 25)
        ld_v8(1)
        ld_b8(25, 33)
        ld_v8(2)
        ld_k8(1)
        ld_v8(3)
        ld_q8(1)
        ld_k8(2)
        ld_q8(2)
        ld_k8(3)
        ld_q8(3)
        nc.sync.dma_start(out=m_sb, in_=mres.rearrange("(c p) s -> p c s", p=P))
        nc.sync.dma_start(out=wc_sb, in_=Wc8[:, :, :, :])

        # residual ident: diag = 1/G_OUT = 2^20 (exact in bf16)
        identB = consts.tile([P, P], BF16)
        make_identity(nc, identB)
        identR = consts.tile([P, P], BF16)
        nc.scalar.mul(identR, identB, 1.0 / G_OUT)

        attnT8 = consts.tile([P, NPAIR, TOK], F8E4)

        PV_DELAY = 2

        for j in range(NPAIR):
            g, half = j // 2, j % 2
            pv = pspv.tile([96, 2, TOK], F32, tag="pv")

            def issue_pv(kcp, ptile):
                nc.tensor.matmul(
                    pv[:, 0, :],
                    lhsT=v8sb[:, 2 * kcp : 2 * kcp + 2, 130 * j : 130 * j + 96],
                    rhs=ptile[:, 0, :, :],
                    start=(kcp == 0),
                    stop=(kcp == NKCP - 1),
                    perf_mode=DR,
                )
                nc.tensor.matmul(
                    pv[:, 1, :],
                    lhsT=v8sb[:, 2 * kcp : 2 * kcp + 2, 130 * j + 65 : 130 * j + 161],
                    rhs=ptile[:, 1, :, :],
                    start=(kcp == 0),
                    stop=(kcp == NKCP - 1),
                    perf_mode=DR,
                )

            pending = []
            for kcp in range(NKCP):
                ptile = ppool.tile([P, 2, 2, TOK], F8E5, tag="pt")
                for sub in range(2):
                    kc = 2 * kcp + sub
                    is_act = kc in ACT_SET
                    ps = psqk.tile([P, 2, TOK], F32, tag="s")
                    for h in range(2):
                        base = 64 * half + 32 * h
                        nc.tensor.matmul(
                            ps[:, h, :],
                            lhsT=k8sb[base : base + 32, g, :, kc * P : (kc + 1) * P],
                            rhs=q8sb[base : base + 32, g, :, :],
                            start=True,
                            stop=not is_act,
                            perf_mode=DR,
                            tile_position=(base, 0),
                        )
                        if is_act:
                            nc.tensor.matmul(
                                ps[:, h, :],
                                lhsT=identg,
                                rhs=b8sb[:, kc : kc + 2, :],
                                start=False,
                                stop=True,
                                perf_mode=DR,
                                skip_group_check=True,
                            )
                    if is_act:
                        nc.scalar.activation(
                            out=ptile[:, :, sub, :].bitcast(I8),
                            in_=ps,
                            func=AF.Identity,
                            bias=shift_ap,
                            scale=1.0,
                        )
                    else:
                        nc.vector.scalar_tensor_tensor(
                            out=ptile[:, :, sub, :].bitcast(I8),
                            in0=ps,
                            scalar=SHIFT,
                            in1=b8sb[:, kc : kc + 1, :].rearrange(
                                "p (o k) n -> p o k n", o=1
                            )[:, 0, :, :].broadcast_to([P, 2, TOK]),
                            op0=ALU.add,
                            op1=ALU.add,
                        )
                pending.append((kcp, ptile))
                if len(pending) > PV_DELAY:
                    issue_pv(*pending.pop(0))
            for item in pending:
                issue_pv(*item)

            # evac pv (both halves in one op), reciprocal of denominators,
            # broadcast + normalize off the critical path on Pool (DVE for j=7)
            st = small.tile([65, 2, TOK], BF16, tag="st")
            nc.scalar.copy(st, pv[0:65, :, :])
            rec = small.tile([1, 2, TOK], BF16, tag="rec")
            with nc.allow_low_precision("bf16 softmax denominators, ~0.4% scale"):
                nc.vector.reciprocal(rec, st[64:65, :, :])
            bcastA = small.tile([64, TOK], BF16, tag="bcastA")
            bcastB = small.tile([64, TOK], BF16, tag="bcastB")
            nc.gpsimd.partition_broadcast(bcastA, rec[:, 0, :], channels=64)
            nc.gpsimd.partition_broadcast(bcastB, rec[:, 1, :], channels=64)
            eng = nc.vector if j == NPAIR - 1 else nc.gpsimd
            eng.tensor_tensor(
                out=attnT8[0:64, j, :], in0=st[0:64, 0, :], in1=bcastA,
                op=ALU.mult,
            )
            eng.tensor_tensor(
                out=attnT8[64:128, j, :], in0=st[0:64, 1, :], in1=bcastB,
                op=ALU.mult,
            )

        # output projection (fp8 DR) with residual accumulated via bf16 ident
        o_sb = consts.tile([P, NTC, N_STATE], BF16)
        for qc in range(NTC):
            ps_o = psqk.tile([P, 2, TOK], F32, tag="s")
            po = ps_o.rearrange("p a b -> p (a b)")
            for pc in range(2):
                for u in range(4):
                    nc.tensor.matmul(
                        ps_o[:, pc, :],
                        lhsT=attnT8[:, 2 * u : 2 * u + 2, qc * P : (qc + 1) * P],
                        rhs=wc_sb[:, u, :, pc * TOK : (pc + 1) * TOK],
                        start=(u == 0),
                        stop=False,
                        perf_mode=DR,
                    )
                nc.tensor.matmul(
                    ps_o[:, pc, :],
                    lhsT=identR,
                    rhs=m_sb[:, qc, pc * TOK : (pc + 1) * TOK],
                    start=False,
                    stop=True,
                    skip_group_check=True,
                )
            if qc % 2 == 0:
                nc.scalar.mul(o_sb[:, qc, :], po, G_OUT)
            else:
                nc.vector.tensor_scalar(
                    out=o_sb[:, qc, :], in0=po, scalar1=G_OUT, scalar2=None,
                    op0=ALU.mult,
                )
            nc.sync.dma_start(
                out=o_out.rearrange("(c p) s -> p c s", p=P)[:, qc, :],
                in_=o_sb[:, qc, :],
            )
    nc.compile()
    return nc


_NC_CACHE = {}


def _get_nc(which):
    if which not in _NC_CACHE:
        _NC_CACHE[which] = _build_phase1() if which == 1 else _build_phase2()
    return _NC_CACHE[which]


def _perm_cols():
    """Column permutation for q/k weights: per pair j, [hA d0:32 | hB d0:32 |
    hA d32:64 | hB d32:64]."""
    order = []
    for j in range(NSC):
        hA, hB = 2 * j, 2 * j + 1
        order.extend(range(hA * 64, hA * 64 + 32))
        order.extend(range(hB * 64, hB * 64 + 32))
        order.extend(range(hA * 64 + 32, hA * 64 + 64))
        order.extend(range(hB * 64 + 32, hB * 64 + 64))
    return np.array(order)


def _w_dr_layout(w8):
    """[1024, C] -> [128, 4, 2, C] DoubleRow lhsT layout."""
    return np.ascontiguousarray(
        w8.reshape(4, 2, P, -1).transpose(2, 0, 1, 3)
    )


def kernel(m, bias, gamma, beta, Wq, bq, Wk, Wv, bv, Wc, bc, _want_timing=None):
    m = np.asarray(m, dtype=np.float32).reshape(N_CTX, N_STATE)
    bias = np.asarray(bias, np.float32)
    gamma = np.asarray(gamma, np.float32)
    beta = np.asarray(beta, np.float32)
    Wq = np.asarray(Wq, np.float32)
    Wk = np.asarray(Wk, np.float32)
    Wv = np.asarray(Wv, np.float32)
    Wc = np.asarray(Wc, np.float32)
    bq = np.asarray(bq, np.float32)
    bv = np.asarray(bv, np.float32)
    bc = np.asarray(bc, np.float32)

    m_pad = np.zeros((T_PAD, N_STATE), np.float32)
    m_pad[:N_CTX] = m

    # fold gamma into weights, beta into biases; bv and bc fold into residual
    Wqf = gamma[:, None] * Wq
    Wkf = gamma[:, None] * Wk
    Wvf = gamma[:, None] * Wv
    bqf = bq + beta @ Wq
    # beta@Wk shifts all logits of a query equally -> softmax invariant; drop.
    # beta@Wv + bv shift attention output -> fold into residual with bc.
    perm = _perm_cols()
    Wq8 = _w_dr_layout((LAM_W * Wqf[:, perm]).astype(E4NP))
    Wk8 = _w_dr_layout((LAM_W * Wkf[:, perm]).astype(E4NP))
    Wv8 = _w_dr_layout((LAM_W * Wvf).astype(E4NP))
    bqs = (LAM_Q * bqf[perm]).astype(np.float32)
    Wc8 = _w_dr_layout((LAM_WC * Wc).astype(E4NP))
    mres_full = (m_pad + (bc + (bv + beta @ Wv) @ Wc)[None, :]).astype(BFNP)

    import sys as _sys

    def _log(*a):
        print("[kernel]", *a, file=_sys.stderr, flush=True)

    nc1 = _get_nc(1)
    _log("phase1 built")
    in_maps1 = []
    for c in range(N_CORES):
        in_maps1.append(
            {
                "m_blk": np.ascontiguousarray(
                    m_pad[c * TOK : (c + 1) * TOK].astype(BFNP)
                ),
                "Wq8": Wq8,
                "Wk8": Wk8,
                "Wv8": Wv8,
                "bqs": bqs,
            }
        )
    res1 = run_bass_kernel_spmd(nc1, in_maps1, core_ids=list(range(N_CORES)))
    _log("phase1 done")

    q8_blks = [r["q8_out"] for r in res1.results]
    k8_full = np.concatenate([r["k8_out"] for r in res1.results], axis=3)
    v8_full = np.concatenate([r["v8_out"] for r in res1.results], axis=0)
    v8_full[N_CTX:] = 0  # pad tokens carry no value

    # v8 pair-tile layout [128, 32, VW] with denominator columns
    v8f = v8_full.astype(np.float32).reshape(NKC, P, N_HEADS, D_HEAD)
    v8h = np.zeros((P, NKC, VW), np.float32)
    for j in range(NPAIR):
        v8h[:, :, 130 * j : 130 * j + 64] = v8f[:, :, 2 * j].transpose(1, 0, 2)
        v8h[:, :, 130 * j + 65 : 130 * j + 129] = v8f[:, :, 2 * j + 1].transpose(1, 0, 2)
        v8h[:, :, 130 * j + 64] = ONES_VAL
        v8h[:, :, 130 * j + 129] = ONES_VAL
    # zero the denominator contribution of padded keys
    keyidx = (np.arange(NKC)[None, :] * P + np.arange(P)[:, None])  # [p, kc]
    padmask = keyidx >= N_CTX
    for j in range(NPAIR):
        v8h[:, :, 130 * j + 64][padmask] = 0.0
        v8h[:, :, 130 * j + 129][padmask] = 0.0
    v8h8 = v8h.astype(E4NP)
    v8_dr = np.ascontiguousarray(
        v8h8.reshape(P, 4, NKC // 4, VW).transpose(1, 0, 2, 3)
    )

    biasT = np.ascontiguousarray(bias.T)  # [k, q]

    nc2 = _get_nc(2)
    _log("phase2 built")
    in_maps2 = []
    for c in range(N_CORES):
        qs = slice(c * TOK, (c + 1) * TOK)
        b8 = np.zeros((NKC + 1, P, TOK), E4NP)
        b8[:NKC] = (ALPHA * biasT[:, qs]).reshape(NKC, P, TOK).astype(E4NP)
        in_maps2.append(
            {
                "q8_in": np.ascontiguousarray(q8_blks[c]),
                "k8_in": k8_full,
                "v8_in": v8_dr,
                "b8_in": b8,
                "mres": np.ascontiguousarray(mres_full[qs]),
                "Wc8": Wc8,
            }
        )
    res2 = run_bass_kernel_spmd(nc2, in_maps2, core_ids=list(range(N_CORES)))
    _log("phase2 done")
    o = np.concatenate([r["o_out"] for r in res2.results], axis=0)[:N_CTX]
    if _want_timing is not None:
        _want_timing["res1"] = res1
        _want_timing["res2"] = res2
    return o.reshape(1, N_CTX, N_STATE).astype(np.float32)
